# revision 41
# baseline (speedup 1.0000x reference)
"""Causal single-head attention (B=4, S=4096, D=1024) on 8 TRN2 NeuronCores.

Sharding: core = (batch b, half h).  Each core computes attention output for
2048 queries of one batch: query chunks {0,3,4,7} (h=0) or {1,2,5,6} (h=1) of
8x512, which balances causal work.  Each core projects K^T/V for its full
batch (Q projections zippered in between the chunks); K^T and V both live
entirely in SBUF as fp8 (V is 4MB = 32KB/partition), so phase 2 needs no DMA
except the output.

All heavy matmuls run fp8e4m3 with perf_mode=DoubleRow (2 contraction slabs
per pass), except a small fp16 "island" covering keys 0..511 for slot-0
queries (chunks 0/1): early causal queries have peaked softmax, so fp8
quantization of scores/V would land directly on the output there.  The
island chunk (c=0) is projected LAST so its fp16 weights never gate startup.

  K^T/Q^T/V projections:  psum = sum_d2 WT[d2,:,:128].T @ x^T[d2,:,:]  (fp8 DR)
  scores^T[k,q]        :  psum = sum_o2 KT[o2,:,k128].T @ QT[o2,:,q512] (fp8 DR)
  P = exp(s*scale) * causal_mask   (mask = (iota_k - iota_q) <= a[slot,j]);
      causally-full tiles skip the mask: ACT writes exp straight to fp8
  den[1,q]             :  DVE-accumulate P tiles, then ones[k,1].T @ acc;
                          1/den = exp(-ln(den)) on ACT
  ctx^T[o,q]           :  one psum group per (slot,o) over all k-pairs
                          (fp8 DR); ft = psum * (1/den) -> fp16 out
"""

import sys

for _p in ("/opt/trn_rl_repo",):
    if _p not in sys.path:
        sys.path.insert(0, _p)

import numpy as np

B, S, D = 4, 4096, 1024
P = 128
CH = 512                       # query chunk
NSLOT = 4                      # chunks per core
NQ = NSLOT * CH                # queries per core
NK = [8, 16, 24, 32]           # k-tiles per slot (uniform across cores)
SLOTBASE = [0, 8, 24, 48]      # amat column base per slot
CHUNKS_H = [[0, 3, 4, 7], [1, 2, 5, 6]]
SCALE = 1.0 / 32.0             # 1/sqrt(D)

_PROGRAM = None


def _build_program():
    import concourse.bass as bass
    import concourse.tile as tile
    import concourse.mybir as mybir
    from concourse import bacc
    from concourse.bass import ds, ts

    f32 = mybir.dt.float32
    f16 = mybir.dt.float16
    f8 = mybir.dt.float8e4
    DR = mybir.MatmulPerfMode.DoubleRow

    nc = bacc.Bacc(trn_type="TRN2", target_bir_lowering=False, debug=False,
                   num_devices=8)

    # positions: [chunk 1, excl0, excl1, excl2] (excl = 2,3,4 on h=0 /
    # 5,6,7 on h=1); chunk 0 arrives as fp16 via xc16
    xT8 = nc.declare_dram_parameter("xT8", [4, P, 8, CH], f8, isOutput=False)
    xc16d = nc.declare_dram_parameter("xc16", [P, 8, 256], f16, isOutput=False)
    xh8d = nc.declare_dram_parameter("xh8", [P, 8, 256], f8, isOutput=False)
    xq8d = nc.declare_dram_parameter("xq8", [3, P, 8, CH], f8, isOutput=False)
    xq16d = nc.declare_dram_parameter("xq16", [P, 8, CH], f16, isOutput=False)
    wq8d = nc.declare_dram_parameter("wq8", [P, 8, D], f8, isOutput=False)
    wk8d = nc.declare_dram_parameter("wk8", [P, 8, D], f8, isOutput=False)
    wv8d = nc.declare_dram_parameter("wv8", [P, 8, D], f8, isOutput=False)
    wq16d = nc.declare_dram_parameter("wq16", [P, 8, D], f16, isOutput=False)
    wk16d = nc.declare_dram_parameter("wk16", [P, 8, D], f16, isOutput=False)
    wv16d = nc.declare_dram_parameter("wv16", [P, 8, D], f16, isOutput=False)
    amat = nc.declare_dram_parameter("amat", [P, 80], f16, isOutput=False)
    dmat = nc.declare_dram_parameter("dmat", [P, CH], f16, isOutput=False)
    ones_k = nc.declare_dram_parameter("ones_k", [P, 1], f16, isOutput=False)
    ones_r = nc.declare_dram_parameter("ones_r", [1, P], f16, isOutput=False)
    outT = nc.declare_dram_parameter("outT", [D, NQ], f16, isOutput=True)

    H = S // 4  # 1024: columns per resident K^T piece

    # pairwise exchange buffers for the exclusive-chunk K/V split.
    # Section = 4KB/partition: K chunk [P, 8, CH] or V chunk [P, 2, 2, D].
    SEC = 8 * CH
    cc_a_in = nc.dram_tensor("cc_a_in", [2, 2, P, SEC], f8)
    cc_a_out = nc.dram_tensor("cc_a_out", [2, 2, 2, P, SEC], f8)
    cc_b_in = nc.dram_tensor("cc_b_in", [1, 2, P, SEC], f8)
    cc_b_out = nc.dram_tensor("cc_b_out", [2, 1, 2, P, SEC], f8)
    CC_GROUPS = [[0, 1], [2, 3], [4, 5], [6, 7]]
    CH_A = [[2, 3], [5, 6]]   # cc_a sections: rank -> chunk ids
    CH_B = [4, 7]             # cc_b: rank -> chunk id

    Exp = mybir.ActivationFunctionType.Exp
    Ln = mybir.ActivationFunctionType.Ln
    is_le = mybir.AluOpType.is_le
    mult = mybir.AluOpType.mult

    with tile.TileContext(nc, pool_alloc_mode="queue") as tc:
        with (
            tc.tile_pool(name="kt", bufs=1) as kt_pool,
            tc.tile_pool(name="qt", bufs=1) as qt_pool,
            tc.tile_pool(name="vs", bufs=1) as vs_pool,
            tc.tile_pool(name="const", bufs=1) as const_pool,
        ):
            KTp = [
                kt_pool.tile([P, 8, H], f8, tag=f"kt{i}", name=f"KTp{i}")
                for i in range(4)
            ]
            KT16 = kt_pool.tile([P, 8, 256], f16, tag="kt16", name="KT16")
            QTs = [
                qt_pool.tile([P, 8, CH], f8, tag=f"qt{i}", name=f"QTs{i}")
                for i in range(NSLOT)
            ]
            QT16 = qt_pool.tile([P, 8, CH], f16, tag="qt16", name="QT16")
            # V resident in SBUF: fp8 k-pair layout + fp16 island (keys 0..511)
            vsb = vs_pool.tile([P, 16, 2, D], f8, tag="vsb", name="vsb")
            v16 = vs_pool.tile([P, 2, D], f16, tag="v16", name="v16")
            dmat_sb = const_pool.tile([P, CH], f16, tag="dmat")
            amat_sb = const_pool.tile([P, 80], f16, tag="amat")
            ones_k_sb = const_pool.tile([P, 1], f16, tag="onesk")
            ones_r_sb = const_pool.tile([1, P], f16, tag="onesr")
            nc.gpsimd.dma_start(out=dmat_sb[:], in_=dmat[:])
            nc.gpsimd.dma_start(out=amat_sb[:], in_=amat[:])
            nc.gpsimd.dma_start(out=ones_k_sb[:], in_=ones_k[:])
            nc.gpsimd.dma_start(out=ones_r_sb[:], in_=ones_r[:])

            # ---------- Phase 0+1: local projections (K, V, Q zippered) ----
            with (
                tc.tile_pool(name="w0", bufs=1) as w_pool,
                tc.tile_pool(name="xc", bufs=2) as x_pool,
                tc.tile_pool(name="xq", bufs=2) as xq_pool,
                tc.tile_pool(name="st", bufs=2) as st_pool,
                tc.tile_pool(name="ps0", bufs=4, space="PSUM") as ps_pool,
            ):
                wk8 = w_pool.tile([P, 8, D], f8, tag="wk8")
                wv8 = w_pool.tile([P, 8, D], f8, tag="wv8")
                wq8 = w_pool.tile([P, 8, D], f8, tag="wq8")
                # wa16 carries wq16 (Q island) then is reloaded with wk16
                wa16 = w_pool.tile([P, 8, D], f16, tag="wa16")
                wv16 = w_pool.tile([P, 8, D], f16, tag="wv16")
                x16q = w_pool.tile([P, 8, CH], f16, tag="x16q")
                x16c = w_pool.tile([P, 8, 256], f16, tag="x16c")
                xh8 = w_pool.tile([P, 8, 256], f8, tag="xh8")
                # striped initial loads for the first fp8 chunk
                for d2 in range(4):
                    eng = nc.sync if d2 < 2 else nc.scalar
                    eng.dma_start(
                        out=wk8[:, ds(2 * d2, 2), :],
                        in_=wk8d[:, ds(2 * d2, 2), :],
                    )


                def load_xq(s):
                    xq = xq_pool.tile([P, 8, CH], f8, tag="xq", name=f"xq{s}")
                    nc.scalar.dma_start(out=xq[:], in_=xq8d[s - 1])
                    return xq

                xq_pending = {}

                def proj_q8(s):
                    xq = xq_pending[s]
                    for o in range(8):
                        ps = ps_pool.tile([P, CH], f32, tag="ps", name="psq")
                        for d2 in range(4):
                            nc.tensor.matmul(
                                ps[:],
                                lhsT=wq8[:, ds(2 * d2, 2), ts(o, P)],
                                rhs=xq[:, ds(2 * d2, 2), :],
                                start=(d2 == 0),
                                stop=(d2 == 3),
                                perf_mode=DR,
                            )
                        nc.vector.tensor_copy(QTs[s][:, o, :], ps[:])

                def proj_q16():
                    # slot-0 Q in fp16 (wa16 = wq16, x16 = xq16), dual-cast
                    for o in range(8):
                        ps = ps_pool.tile([P, CH], f32, tag="ps", name="psq6")
                        for d in range(8):
                            nc.tensor.matmul(
                                ps[:],
                                lhsT=wa16[:, d, ts(o, P)],
                                rhs=x16q[:, d, :],
                                start=(d == 0),
                                stop=(d == 7),
                            )
                        nc.vector.tensor_copy(QT16[:, o, :], ps[:])
                        nc.scalar.copy(QTs[0][:, o, :], ps[:])

                def proj_kv16():
                    # chunk 0: tokens 0..255 in fp16 (wa16 = wk16, x16c),
                    # tokens 256..511 in fp8 (xh8) - the island only needs
                    # keys 0..255
                    for o in range(8):
                        ps = ps_pool.tile([P, 256], f32, tag="ps6", name="psk6")
                        for d in range(8):
                            nc.tensor.matmul(
                                ps[:],
                                lhsT=wa16[:, d, ts(o, P)],
                                rhs=x16c[:, d, :],
                                start=(d == 0),
                                stop=(d == 7),
                            )
                        nc.vector.tensor_copy(KT16[:, o, :], ps[:])
                        nc.scalar.copy(KTp[0][:, o, ds(0, 256)], ps[:])
                    for kt_i in range(2):
                        for oh in range(2):
                            ps = ps_pool.tile([P, CH], f32, tag="ps", name="psv6")
                            for d in range(8):
                                nc.tensor.matmul(
                                    ps[:],
                                    lhsT=x16c[:, d, ts(kt_i, P)],
                                    rhs=wv16[:, d, ts(oh, CH)],
                                    start=(d == 0),
                                    stop=(d == 7),
                                )
                            nc.scalar.copy(v16[:, kt_i, ts(oh, CH)], ps[:])
                            nc.vector.tensor_copy(
                                vsb[:, 0, kt_i, ts(oh, CH)], ps[:]
                            )
                    for o in range(8):
                        ps = ps_pool.tile([P, 256], f32, tag="ps6", name="psk6b")
                        for d2 in range(4):
                            nc.tensor.matmul(
                                ps[:],
                                lhsT=wk8[:, ds(2 * d2, 2), ts(o, P)],
                                rhs=xh8[:, ds(2 * d2, 2), :],
                                start=(d2 == 0),
                                stop=(d2 == 3),
                                perf_mode=DR,
                            )
                            if o % 2 == 0 and d2 == 3:
                                nc.vector.tensor_copy(
                                    KTp[0][:, o, ds(256, 256)], ps[:])
                            elif d2 == 3:
                                nc.scalar.copy(
                                    KTp[0][:, o, ds(256, 256)], ps[:])
                    for kt_i in range(2):
                        for oh in range(2):
                            ps = ps_pool.tile([P, CH], f32, tag="ps", name="psv6b")
                            for d2 in range(4):
                                nc.tensor.matmul(
                                    ps[:],
                                    lhsT=xh8[:, ds(2 * d2, 2), ts(kt_i, P)],
                                    rhs=wv8[:, ds(2 * d2, 2), ts(oh, CH)],
                                    start=(d2 == 0),
                                    stop=(d2 == 3),
                                    perf_mode=DR,
                                )
                            nc.scalar.copy(
                                vsb[:, 1, kt_i, ts(oh, CH)], ps[:]
                            )

                # positions: [chunk1, excl0, excl1, excl2, chunk0].  Chunk 0
                # (fp16 island) LAST so its weights stream in while fp8
                # chunks compute.  Exclusive chunks (this core only) are
                # staged to DRAM and AllGathered with the pair core; both
                # cores then read all six sections back into KTp/vsb at
                # uniform addresses.  Q slots zippered after iters 1..4
                # (slot 0 at it==3 so x16/wa16 can be reloaded for chunk 0).
                q_sched = {1: 0, 2: 1, 3: 2, 4: 3}
                xc_tiles = {}

                def get_xc(i):
                    if i not in xc_tiles and i < 4:
                        xc_tiles[i] = x_pool.tile([P, 8, CH], f8, tag="xc",
                                                  name=f"xc{i}")
                        if i == 0:
                            for sp in range(4):
                                eng = nc.gpsimd if sp % 2 == 0 else nc.sync
                                eng.dma_start(
                                    out=xc_tiles[i][:, ds(sp * 2, 2), :],
                                    in_=xT8[0][:, ds(sp * 2, 2), :],
                                )
                        else:
                            nc.sync.dma_start(out=xc_tiles[i][:], in_=xT8[i])
                    return xc_tiles.get(i)

                for it in range(5):
                    if it == 4:
                        proj_kv16()
                    else:
                        excl = it >= 1
                        xc = get_xc(it)
                        if excl:
                            ktmp = st_pool.tile([P, 8, CH], f8, tag="ktmp",
                                                name=f"ktmp{it}")
                            vtmp = st_pool.tile([P, 2, 2, D], f8, tag="vtmp",
                                                name=f"vtmp{it}")
                        for o in range(8):
                            ps = ps_pool.tile([P, CH], f32, tag="ps", name="psk")
                            for d2 in range(4):
                                nc.tensor.matmul(
                                    ps[:],
                                    lhsT=wk8[:, ds(2 * d2, 2), ts(o, P)],
                                    rhs=xc[:, ds(2 * d2, 2), :],
                                    start=(d2 == 0),
                                    stop=(d2 == 3),
                                    perf_mode=DR,
                                )
                            kdst = (
                                ktmp[:, o, :] if excl
                                else KTp[0][:, o, ds(CH, CH)]
                            )
                            if o % 2 == 0:
                                nc.vector.tensor_copy(kdst, ps[:])
                            else:
                                nc.scalar.copy(kdst, ps[:])
                        get_xc(it + 1)
                        if it == 0:
                            # deferred loads, enqueued between chunk-1's K and
                            # V work (wv8 must precede the V copies in the
                            # ACT queue to avoid a trigger deadlock)
                            for d2 in range(4):
                                nc.scalar.dma_start(
                                    out=wv8[:, ds(2 * d2, 2), :],
                                    in_=wv8d[:, ds(2 * d2, 2), :],
                                )
                            nc.scalar.dma_start(out=wq8[:], in_=wq8d[:])
                            xq_pending[1] = load_xq(1)
                            xq_pending[2] = load_xq(2)
                            for sp in range(4):
                                eng = nc.gpsimd if sp % 2 == 0 else nc.scalar
                                eng.dma_start(
                                    out=wa16[:, ds(2 * sp, 2), :],
                                    in_=wq16d[:, ds(2 * sp, 2), :],
                                )
                            nc.sync.dma_start(out=x16q[:], in_=xq16d[:])
                        for kt_i in range(4):
                            for oh in range(2):
                                ps = ps_pool.tile([P, CH], f32, tag="ps", name="psv")
                                for d2 in range(4):
                                    nc.tensor.matmul(
                                        ps[:],
                                        lhsT=xc[:, ds(2 * d2, 2), ts(kt_i, P)],
                                        rhs=wv8[:, ds(2 * d2, 2), ts(oh, CH)],
                                        start=(d2 == 0),
                                        stop=(d2 == 3),
                                        perf_mode=DR,
                                    )
                                vdst = (
                                    vtmp[:, kt_i // 2, kt_i % 2, ts(oh, CH)]
                                    if excl
                                    else vsb[:, 2 + kt_i // 2, kt_i % 2,
                                             ts(oh, CH)]
                                )
                                nc.scalar.copy(vdst, ps[:])
                        if excl:
                            li = it - 1
                            cin = cc_a_in if li < 2 else cc_b_in
                            sec = li if li < 2 else 0
                            nc.scalar.dma_start(out=cin[sec, 0], in_=ktmp[:])
                            nc.scalar.dma_start(out=cin[sec, 1], in_=vtmp[:])
                            if li == 1:
                                nc.gpsimd.collective_compute(
                                    "AllGather", mybir.AluOpType.bypass,
                                    replica_groups=CC_GROUPS,
                                    ins=[cc_a_in[:]], outs=[cc_a_out[:]],
                                )
                            elif li == 2:
                                nc.gpsimd.collective_compute(
                                    "AllGather", mybir.AluOpType.bypass,
                                    replica_groups=CC_GROUPS,
                                    ins=[cc_b_in[:]], outs=[cc_b_out[:]],
                                )
                    sq = q_sched.get(it)
                    if sq is not None:
                        if sq == 0:
                            proj_q16()
                            # island loads (consumed at it==4); wa16 reuses
                            # wq16's tile (WAR resolves as proj_q16 drains)
                            for sp in range(4):
                                eng = nc.sync if sp % 2 == 0 else nc.gpsimd
                                eng.dma_start(
                                    out=wa16[:, ds(2 * sp, 2), :],
                                    in_=wk16d[:, ds(2 * sp, 2), :],
                                )
                            nc.sync.dma_start(out=x16c[:], in_=xc16d[:])
                            nc.gpsimd.dma_start(out=xh8[:], in_=xh8d[:])
                        else:
                            proj_q8(sq)
                            if sq == 1:
                                for sp in range(4):
                                    nc.scalar.dma_start(
                                        out=wv16[:, ds(2 * sp, 2), :],
                                        in_=wv16d[:, ds(2 * sp, 2), :],
                                    )
                            if sq == 2:
                                xq_pending[3] = load_xq(3)
                # read the gathered exclusive sections back (both ranks'),
                # at chunk-uniform addresses; DMAs wait on the collectives
                qi = 0
                for r in range(2):
                    for s2 in range(2):
                        cid = CH_A[r][s2]
                        eng = nc.gpsimd if qi % 2 == 0 else nc.sync
                        eng.dma_start(
                            out=KTp[cid // 2][:, :, ds((cid % 2) * CH, CH)],
                            in_=cc_a_out[r, s2, 0],
                        )
                        eng.dma_start(
                            out=vsb[:, ds(2 * cid, 2), :, :],
                            in_=cc_a_out[r, s2, 1],
                        )
                        qi += 1
                for r in range(2):
                    cid = CH_B[r]
                    eng = nc.gpsimd if qi % 2 == 0 else nc.sync
                    eng.dma_start(
                        out=KTp[cid // 2][:, :, ds((cid % 2) * CH, CH)],
                        in_=cc_b_out[r, 0, 0],
                    )
                    eng.dma_start(
                        out=vsb[:, ds(2 * cid, 2), :, :],
                        in_=cc_b_out[r, 0, 1],
                    )
                    qi += 1

            # ---------------- Phase 2: attention ---------------------------
            # Per slot: all score k-tiles (P tiles + den accumulate), then
            # ctx as ONE psum group per o; ft multiplies read psum directly.
            # Tiles causally full on BOTH halves skip the mask STT: ACT
            # writes exp() straight into the fp8 P pair tile.
            CLEAN_NK = [
                min(4 * CHUNKS_H[0][s], 4 * CHUNKS_H[1][s], NK[s])
                for s in range(NSLOT)
            ]
            with (
                tc.tile_pool(name="pt", bufs=20) as p_pool,
                tc.tile_pool(name="p6", bufs=4) as p16_pool,
                tc.tile_pool(name="et", bufs=3) as e_pool,
                tc.tile_pool(name="fo", bufs=6) as f_pool,
                tc.tile_pool(name="dsb", bufs=2) as den_pool,
                tc.tile_pool(name="cx0", bufs=2) as ctx0_pool,
                tc.tile_pool(name="pss", bufs=4, space="PSUM") as s_ps_pool,
                tc.tile_pool(name="psc", bufs=2, space="PSUM") as c_ps_pool,
                tc.tile_pool(name="psd", bufs=1, space="PSUM") as d_ps_pool,
                tc.tile_pool(name="psb", bufs=1, space="PSUM") as b_ps_pool,
            ):
                for slot in range(NSLOT):
                    nk = NK[slot]
                    acc = den_pool.tile([P, CH], f16, tag="acc", name="acc")
                    pt16s = []
                    pt2s = []
                    cur_pt2 = None
                    for j in range(nk):
                        island = slot == 0 and j < 2
                        clean = j < CLEAN_NK[slot]
                        sps = s_ps_pool.tile([P, CH], f32, name="sps")
                        if island:
                            for o in range(8):
                                nc.tensor.matmul(
                                    sps[:],
                                    lhsT=KT16[:, o, ds(j * P, P)],
                                    rhs=QT16[:, o, :],
                                    start=(o == 0),
                                    stop=(o == 7),
                                )
                        else:
                            for o2 in range(4):
                                nc.tensor.matmul(
                                    sps[:],
                                    lhsT=KTp[j // 8][
                                        :, ds(2 * o2, 2), ds((j % 8) * P, P)
                                    ],
                                    rhs=QTs[slot][:, ds(2 * o2, 2), :],
                                    start=(o2 == 0),
                                    stop=(o2 == 3),
                                    perf_mode=DR,
                                )
                        if not island and j % 2 == 0:
                            cur_pt2 = p_pool.tile([P, 2, CH], f8, tag="pt",
                                                  name="pt2")
                        col = SLOTBASE[slot] + j
                        if island:
                            et = e_pool.tile([P, CH], f16, tag="et", name="et")
                            nc.scalar.activation(et[:], sps[:], Exp, scale=SCALE)
                            pt = p16_pool.tile([P, CH], f16, tag="pt16",
                                               name="pt16")
                            nc.vector.scalar_tensor_tensor(
                                out=pt[:],
                                in0=dmat_sb[:],
                                scalar=amat_sb[:, ds(col, 1)],
                                in1=et[:],
                                op0=is_le,
                                op1=mult,
                            )
                            pt16s.append(pt)
                            ptv = pt[:]
                        elif clean:
                            nc.scalar.activation(
                                cur_pt2[:, j % 2, :], sps[:], Exp, scale=SCALE
                            )
                            ptv = cur_pt2[:, j % 2, :]
                        else:
                            et = e_pool.tile([P, CH], f16, tag="et", name="et")
                            nc.scalar.activation(et[:], sps[:], Exp, scale=SCALE)
                            nc.vector.scalar_tensor_tensor(
                                out=cur_pt2[:, j % 2, :],
                                in0=dmat_sb[:],
                                scalar=amat_sb[:, ds(col, 1)],
                                in1=et[:],
                                op0=is_le,
                                op1=mult,
                            )
                            ptv = cur_pt2[:, j % 2, :]
                        if not island and j % 2 == 1:
                            pt2s.append(cur_pt2)
                        if j == 0:
                            nc.vector.tensor_copy(acc[:], ptv)
                        else:
                            nc.vector.tensor_add(acc[:], acc[:], ptv)
                    # den -> 1/den = exp(-ln(den)) -> broadcast; the o=0 ctx
                    # group is emitted between den and bps so the PE never
                    # waits on the ACT chain.
                    dps = d_ps_pool.tile([1, CH], f32, name="dps")
                    nc.tensor.matmul(
                        dps[:], lhsT=ones_k_sb[:], rhs=acc[:], start=True,
                        stop=True,
                    )
                    lden = f_pool.tile([1, CH], f32, tag="lden", name="lden")
                    nc.scalar.activation(lden[:], dps[:], Ln)
                    den_sb = f_pool.tile([1, CH], f16, tag="den", name="den")
                    nc.scalar.activation(den_sb[:], lden[:], Exp, scale=-1.0)

                    n_mm = len(pt16s) + len(pt2s)

                    def ctx_group(o, slot=slot, n_mm=n_mm, pt16s=pt16s,
                                  pt2s=pt2s):
                        cps = c_ps_pool.tile([P, CH], f32, name="cps")
                        idx = 0
                        for jj, pt in enumerate(pt16s):
                            nc.tensor.matmul(
                                cps[:],
                                lhsT=v16[:, jj, ts(o, P)],
                                rhs=pt[:],
                                start=(idx == 0),
                                stop=(idx == n_mm - 1),
                            )
                            idx += 1
                        p0 = 1 if slot == 0 else 0
                        for pi, pt2 in enumerate(pt2s):
                            nc.tensor.matmul(
                                cps[:],
                                lhsT=vsb[:, p0 + pi, :, ts(o, P)],
                                rhs=pt2[:],
                                start=(idx == 0),
                                stop=(idx == n_mm - 1),
                                perf_mode=DR,
                            )
                            idx += 1
                        return cps

                    # flush-mode for every slot: ctx groups flush to SBUF
                    # via DVE (no rec dependency, PSUM frees in ~0.7us) ->
                    # pure PE cover for the den->bps ACT chain AND c_ps can
                    # drop to 2, freeing a PSUM bank for a deeper score ring
                    ctx_sb = ctx0_pool.tile([P, 8, CH], f32, tag="ctx0",
                                            name="ctx0")
                    for o in range(8):
                        cps = ctx_group(o)
                        nc.vector.tensor_copy(ctx_sb[:, o, :], cps[:])
                        if o == 2:
                            bps = b_ps_pool.tile([P, CH], f32, name="bps")
                            nc.tensor.matmul(
                                bps[:], lhsT=ones_r_sb[:], rhs=den_sb[:],
                                start=True, stop=True,
                            )
                            rec = f_pool.tile([P, CH], f16, tag="rec",
                                              name="rec")
                            nc.scalar.copy(rec[:], bps[:])
                    for o in range(8):
                        ft = f_pool.tile([P, CH], f16, tag="ft", name="ft")
                        nc.vector.tensor_mul(ft[:], ctx_sb[:, o, :], rec[:])
                        eng = nc.sync if o % 2 == 0 else nc.scalar
                        eng.dma_start(
                            out=outT[ds(o * P, P), ts(slot, CH)], in_=ft[:]
                        )

    nc.compile()
    return nc


def _get_program():
    global _PROGRAM
    if _PROGRAM is None:
        _PROGRAM = _build_program()
    return _PROGRAM


def _make_in_maps(x, W_query, W_key, W_value):
    import ml_dtypes

    f8 = ml_dtypes.float8_e4m3

    xT = np.ascontiguousarray(
        np.asarray(x, dtype=np.float32).transpose(0, 2, 1).astype(np.float16)
    )
    xT8 = xT.astype(f8)

    def tile_w(w, dt):
        # [d, o] -> [p, d_slab, o]
        wt = np.asarray(w, dtype=np.float32).T.astype(np.float16).astype(dt)
        return np.ascontiguousarray(wt.reshape(8, P, D).transpose(1, 0, 2))

    def tile_x(xt, nch):
        # [d, s] -> [chunk, p, d_slab, s_off]
        return np.ascontiguousarray(
            xt.reshape(8, P, nch, CH).transpose(2, 1, 0, 3)
        )

    w8 = {k: tile_w(w, f8) for k, w in
          (("wq8", W_query), ("wk8", W_key), ("wv8", W_value))}
    w16 = {k: tile_w(w, np.float16) for k, w in
           (("wq16", W_query), ("wk16", W_key), ("wv16", W_value))}
    dmat = (
        np.arange(P, dtype=np.float32)[:, None]
        - np.arange(CH, dtype=np.float32)[None, :]
    )
    dmat = np.ascontiguousarray(dmat.astype(np.float16))
    amat_h = []
    for h in range(2):
        a = np.zeros((P, 80), np.float16)
        for slot in range(NSLOT):
            cid = CHUNKS_H[h][slot]
            for j in range(NK[slot]):
                a[:, SLOTBASE[slot] + j] = CH * cid - P * j
        amat_h.append(a)
    ones_k = np.ones((P, 1), np.float16)
    ones_r = np.ones((1, P), np.float16)

    in_maps = []
    for core in range(8):
        b, h = core // 2, core % 2
        chunks = CHUNKS_H[h]
        xq8 = np.stack(
            [tile_x(xT8[b][:, c * CH : (c + 1) * CH], 1)[0] for c in chunks[1:]]
        )
        xq16 = tile_x(
            xT[b][:, chunks[0] * CH : (chunks[0] + 1) * CH], 1
        )[0]
        # projection positions: chunk 1 then this core's exclusive chunks
        pos_chunks = [1] + ([2, 3, 4] if h == 0 else [5, 6, 7])
        xT8_core = np.stack(
            [tile_x(xT8[b][:, c * CH : (c + 1) * CH], 1)[0] for c in pos_chunks]
        )
        in_maps.append(
            {
                "xT8": xT8_core,
                "xc16": np.ascontiguousarray(
                    xT[b][:, :256].reshape(8, P, 256).transpose(1, 0, 2)
                ),
                "xh8": np.ascontiguousarray(
                    xT8[b][:, 256:512].reshape(8, P, 256).transpose(1, 0, 2)
                ),
                "xq8": xq8,
                "xq16": xq16,
                **w8,
                **w16,
                "amat": amat_h[h],
                "dmat": dmat,
                "ones_k": ones_k,
                "ones_r": ones_r,
            }
        )
    return in_maps


def _assemble(results):
    out = np.empty((B, S, D), np.float32)
    for core in range(8):
        b, h = core // 2, core % 2
        oT = np.asarray(results[core]["outT"]).astype(np.float32)  # [D, NQ]
        for slot, c in enumerate(CHUNKS_H[h]):
            out[b, c * CH : (c + 1) * CH, :] = oT[:, slot * CH : (slot + 1) * CH].T
    return out


def run(inputs, trace=False, trace_cores=None):
    """Run the kernel; returns (output, BassKernelResults)."""
    from concourse.bass_utils import run_bass_kernel_spmd

    nc = _get_program()
    in_maps = _make_in_maps(
        inputs["x"], inputs["W_query"], inputs["W_key"], inputs["W_value"]
    )
    kw = {}
    if trace:
        kw = dict(trace=True, trace_cores=trace_cores, stitch_traces=False)
    res = run_bass_kernel_spmd(nc, in_maps, list(range(8)), **kw)
    return _assemble(res.results), res


def kernel(x, W_query, W_key, W_value):
    out, _ = run({"x": x, "W_query": W_query, "W_key": W_key, "W_value": W_value})
    return out


# revision 44
# speedup vs baseline: 1.0275x; 1.0275x over previous
"""Causal single-head attention (B=4, S=4096, D=1024) on 8 TRN2 NeuronCores.

Sharding: core = (batch b, half h).  Each core computes attention output for
2048 queries of one batch: query chunks {0,3,4,7} (h=0) or {1,2,5,6} (h=1) of
8x512, which balances causal work.  Each core projects K^T/V for its full
batch (Q projections zippered in between the chunks); K^T and V both live
entirely in SBUF as fp8 (V is 4MB = 32KB/partition), so phase 2 needs no DMA
except the output.

All heavy matmuls run fp8e4m3 with perf_mode=DoubleRow (2 contraction slabs
per pass), except a small fp16 "island" covering keys 0..511 for slot-0
queries (chunks 0/1): early causal queries have peaked softmax, so fp8
quantization of scores/V would land directly on the output there.  The
island chunk (c=0) is projected LAST so its fp16 weights never gate startup.

  K^T/Q^T/V projections:  psum = sum_d2 WT[d2,:,:128].T @ x^T[d2,:,:]  (fp8 DR)
  scores^T[k,q]        :  psum = sum_o2 KT[o2,:,k128].T @ QT[o2,:,q512] (fp8 DR)
  P = exp(s*scale) * causal_mask   (mask = (iota_k - iota_q) <= a[slot,j]);
      causally-full tiles skip the mask: ACT writes exp straight to fp8
  den[1,q]             :  DVE-accumulate P tiles, then ones[k,1].T @ acc;
                          1/den = exp(-ln(den)) on ACT
  ctx^T[o,q]           :  one psum group per (slot,o) over all k-pairs
                          (fp8 DR); ft = psum * (1/den) -> fp16 out
"""

import sys

for _p in ("/opt/trn_rl_repo",):
    if _p not in sys.path:
        sys.path.insert(0, _p)

import numpy as np

B, S, D = 4, 4096, 1024
P = 128
CH = 512                       # query chunk
NSLOT = 4                      # chunks per core
NQ = NSLOT * CH                # queries per core
NK = [8, 16, 24, 32]           # k-tiles per slot (uniform across cores)
SLOTBASE = [0, 8, 24, 48]      # amat column base per slot
CHUNKS_H = [[0, 3, 4, 7], [1, 2, 5, 6]]
SCALE = 1.0 / 32.0             # 1/sqrt(D)

_PROGRAM = None


def _build_program():
    import concourse.bass as bass
    import concourse.tile as tile
    import concourse.mybir as mybir
    from concourse import bacc
    from concourse.bass import ds, ts

    f32 = mybir.dt.float32
    f16 = mybir.dt.float16
    f8 = mybir.dt.float8e4
    DR = mybir.MatmulPerfMode.DoubleRow

    nc = bacc.Bacc(trn_type="TRN2", target_bir_lowering=False, debug=False,
                   num_devices=8)

    # positions: [chunk 1, excl0, excl1, excl2] (excl = 2,3,4 on h=0 /
    # 5,6,7 on h=1); chunk 0 arrives as fp16 via xc16
    xT8 = nc.declare_dram_parameter("xT8", [4, P, 8, CH], f8, isOutput=False)
    xc16d = nc.declare_dram_parameter("xc16", [P, 8, 256], f16, isOutput=False)
    xh8d = nc.declare_dram_parameter("xh8", [P, 8, 256], f8, isOutput=False)
    xq8d = nc.declare_dram_parameter("xq8", [3, P, 8, CH], f8, isOutput=False)
    xq16d = nc.declare_dram_parameter("xq16", [P, 8, CH], f16, isOutput=False)
    wq8d = nc.declare_dram_parameter("wq8", [P, 8, D], f8, isOutput=False)
    wk8d = nc.declare_dram_parameter("wk8", [P, 8, D], f8, isOutput=False)
    wv8d = nc.declare_dram_parameter("wv8", [P, 8, D], f8, isOutput=False)
    wq16d = nc.declare_dram_parameter("wq16", [P, 8, D], f16, isOutput=False)
    wk16d = nc.declare_dram_parameter("wk16", [P, 8, D], f16, isOutput=False)
    wv16d = nc.declare_dram_parameter("wv16", [P, 8, D], f16, isOutput=False)
    amat = nc.declare_dram_parameter("amat", [P, 80], f16, isOutput=False)
    dmat = nc.declare_dram_parameter("dmat", [P, CH], f16, isOutput=False)
    ones_k = nc.declare_dram_parameter("ones_k", [P, 1], f16, isOutput=False)
    ones_r = nc.declare_dram_parameter("ones_r", [1, P], f16, isOutput=False)
    outT = nc.declare_dram_parameter("outT", [D, NQ], f16, isOutput=True)

    H = S // 4  # 1024: columns per resident K^T piece

    # pairwise exchange buffers for the exclusive-chunk K/V split.
    # Section = 4KB/partition: K chunk [P, 8, CH] or V chunk [P, 2, 2, D].
    SEC = 8 * CH
    cc_a_in = nc.dram_tensor("cc_a_in", [2, 2, P, SEC], f8)
    cc_a_out = nc.dram_tensor("cc_a_out", [2, 2, 2, P, SEC], f8)
    cc_b_in = nc.dram_tensor("cc_b_in", [1, 2, P, SEC], f8)
    cc_b_out = nc.dram_tensor("cc_b_out", [2, 1, 2, P, SEC], f8)
    CC_GROUPS = [[0, 1], [2, 3], [4, 5], [6, 7]]
    CH_A = [[2, 3], [5, 6]]   # cc_a sections: rank -> chunk ids
    CH_B = [4, 7]             # cc_b: rank -> chunk id

    Exp = mybir.ActivationFunctionType.Exp
    Ln = mybir.ActivationFunctionType.Ln
    is_le = mybir.AluOpType.is_le
    mult = mybir.AluOpType.mult

    with tile.TileContext(nc, pool_alloc_mode="queue") as tc:
        with (
            tc.tile_pool(name="kt", bufs=1) as kt_pool,
            tc.tile_pool(name="qt", bufs=1) as qt_pool,
            tc.tile_pool(name="vs", bufs=1) as vs_pool,
            tc.tile_pool(name="const", bufs=1) as const_pool,
        ):
            KTp = [
                kt_pool.tile([P, 8, H], f8, tag=f"kt{i}", name=f"KTp{i}")
                for i in range(4)
            ]
            KT16 = kt_pool.tile([P, 8, 256], f16, tag="kt16", name="KT16")
            QTs = [
                qt_pool.tile([P, 8, CH], f8, tag=f"qt{i}", name=f"QTs{i}")
                for i in range(NSLOT)
            ]
            QT16 = qt_pool.tile([P, 8, CH], f16, tag="qt16", name="QT16")
            # V resident in SBUF: fp8 k-pair layout + fp16 island (keys 0..511)
            vsb = vs_pool.tile([P, 16, 2, D], f8, tag="vsb", name="vsb")
            v16 = vs_pool.tile([P, 2, D], f16, tag="v16", name="v16")
            dmat_sb = const_pool.tile([P, CH], f16, tag="dmat")
            amat_sb = const_pool.tile([P, 80], f16, tag="amat")
            ones_k_sb = const_pool.tile([P, 1], f16, tag="onesk")
            ones_r_sb = const_pool.tile([1, P], f16, tag="onesr")
            nc.gpsimd.dma_start(out=dmat_sb[:], in_=dmat[:])
            nc.gpsimd.dma_start(out=amat_sb[:], in_=amat[:])
            nc.gpsimd.dma_start(out=ones_k_sb[:], in_=ones_k[:])
            nc.gpsimd.dma_start(out=ones_r_sb[:], in_=ones_r[:])

            # ---------- Phase 0+1: local projections (K, V, Q zippered) ----
            with (
                tc.tile_pool(name="w0", bufs=1) as w_pool,
                tc.tile_pool(name="xc", bufs=2) as x_pool,
                tc.tile_pool(name="xq", bufs=2) as xq_pool,
                tc.tile_pool(name="st", bufs=2) as st_pool,
                tc.tile_pool(name="ps0", bufs=4, space="PSUM") as ps_pool,
            ):
                wk8 = w_pool.tile([P, 8, D], f8, tag="wk8")
                wv8 = w_pool.tile([P, 8, D], f8, tag="wv8")
                wq8 = w_pool.tile([P, 8, D], f8, tag="wq8")
                # wa16 carries wq16 (Q island) then is reloaded with wk16
                wa16 = w_pool.tile([P, 8, D], f16, tag="wa16")
                wv16 = w_pool.tile([P, 8, D], f16, tag="wv16")
                x16q = w_pool.tile([P, 8, CH], f16, tag="x16q")
                x16c = w_pool.tile([P, 8, 256], f16, tag="x16c")
                xh8 = w_pool.tile([P, 8, 256], f8, tag="xh8")
                # striped initial loads for the first fp8 chunk
                for d2 in range(4):
                    eng = nc.sync if d2 < 2 else nc.scalar
                    eng.dma_start(
                        out=wk8[:, ds(2 * d2, 2), :],
                        in_=wk8d[:, ds(2 * d2, 2), :],
                    )


                def load_xq(s):
                    xq = xq_pool.tile([P, 8, CH], f8, tag="xq", name=f"xq{s}")
                    nc.scalar.dma_start(out=xq[:], in_=xq8d[s - 1])
                    return xq

                xq_pending = {}

                def proj_q8(s):
                    xq = xq_pending[s]
                    for o in range(8):
                        ps = ps_pool.tile([P, CH], f32, tag="ps", name="psq")
                        for d2 in range(4):
                            nc.tensor.matmul(
                                ps[:],
                                lhsT=wq8[:, ds(2 * d2, 2), ts(o, P)],
                                rhs=xq[:, ds(2 * d2, 2), :],
                                start=(d2 == 0),
                                stop=(d2 == 3),
                                perf_mode=DR,
                            )
                        nc.vector.tensor_copy(QTs[s][:, o, :], ps[:])

                def proj_q16():
                    # slot-0 Q in fp16 (wa16 = wq16, x16 = xq16), dual-cast
                    for o in range(8):
                        ps = ps_pool.tile([P, CH], f32, tag="ps", name="psq6")
                        for d in range(8):
                            nc.tensor.matmul(
                                ps[:],
                                lhsT=wa16[:, d, ts(o, P)],
                                rhs=x16q[:, d, :],
                                start=(d == 0),
                                stop=(d == 7),
                            )
                        nc.vector.tensor_copy(QT16[:, o, :], ps[:])
                        nc.scalar.copy(QTs[0][:, o, :], ps[:])

                def proj_kv16():
                    # chunk 0: tokens 0..255 in fp16 (wa16 = wk16, x16c),
                    # tokens 256..511 in fp8 (xh8) - the island only needs
                    # keys 0..255
                    for o in range(8):
                        ps = ps_pool.tile([P, 256], f32, tag="ps6", name="psk6")
                        for d in range(8):
                            nc.tensor.matmul(
                                ps[:],
                                lhsT=wa16[:, d, ts(o, P)],
                                rhs=x16c[:, d, :],
                                start=(d == 0),
                                stop=(d == 7),
                            )
                        nc.vector.tensor_copy(KT16[:, o, :], ps[:])
                        nc.scalar.copy(KTp[0][:, o, ds(0, 256)], ps[:])
                    for kt_i in range(2):
                        for oh in range(2):
                            ps = ps_pool.tile([P, CH], f32, tag="ps", name="psv6")
                            for d in range(8):
                                nc.tensor.matmul(
                                    ps[:],
                                    lhsT=x16c[:, d, ts(kt_i, P)],
                                    rhs=wv16[:, d, ts(oh, CH)],
                                    start=(d == 0),
                                    stop=(d == 7),
                                )
                            nc.scalar.copy(v16[:, kt_i, ts(oh, CH)], ps[:])
                            nc.vector.tensor_copy(
                                vsb[:, 0, kt_i, ts(oh, CH)], ps[:]
                            )
                    for o in range(8):
                        ps = ps_pool.tile([P, 256], f32, tag="ps6", name="psk6b")
                        for d2 in range(4):
                            nc.tensor.matmul(
                                ps[:],
                                lhsT=wk8[:, ds(2 * d2, 2), ts(o, P)],
                                rhs=xh8[:, ds(2 * d2, 2), :],
                                start=(d2 == 0),
                                stop=(d2 == 3),
                                perf_mode=DR,
                            )
                            if o % 2 == 0 and d2 == 3:
                                nc.vector.tensor_copy(
                                    KTp[0][:, o, ds(256, 256)], ps[:])
                            elif d2 == 3:
                                nc.scalar.copy(
                                    KTp[0][:, o, ds(256, 256)], ps[:])
                    for kt_i in range(2):
                        for oh in range(2):
                            ps = ps_pool.tile([P, CH], f32, tag="ps", name="psv6b")
                            for d2 in range(4):
                                nc.tensor.matmul(
                                    ps[:],
                                    lhsT=xh8[:, ds(2 * d2, 2), ts(kt_i, P)],
                                    rhs=wv8[:, ds(2 * d2, 2), ts(oh, CH)],
                                    start=(d2 == 0),
                                    stop=(d2 == 3),
                                    perf_mode=DR,
                                )
                            nc.scalar.copy(
                                vsb[:, 1, kt_i, ts(oh, CH)], ps[:]
                            )

                # positions: [chunk1, excl0, excl1, excl2, chunk0].  Chunk 0
                # (fp16 island) LAST so its weights stream in while fp8
                # chunks compute.  Exclusive chunks (this core only) are
                # staged to DRAM and AllGathered with the pair core; both
                # cores then read all six sections back into KTp/vsb at
                # uniform addresses.  Q slots zippered after iters 1..4
                # (slot 0 at it==3 so x16/wa16 can be reloaded for chunk 0).
                q_sched = {1: 0, 2: 1, 3: 2, 4: 3}
                xc_tiles = {}

                def get_xc(i):
                    if i not in xc_tiles and i < 4:
                        xc_tiles[i] = x_pool.tile([P, 8, CH], f8, tag="xc",
                                                  name=f"xc{i}")
                        if i == 0:
                            for sp in range(4):
                                eng = nc.gpsimd if sp % 2 == 0 else nc.sync
                                eng.dma_start(
                                    out=xc_tiles[i][:, ds(sp * 2, 2), :],
                                    in_=xT8[0][:, ds(sp * 2, 2), :],
                                )
                        else:
                            nc.sync.dma_start(out=xc_tiles[i][:], in_=xT8[i])
                    return xc_tiles.get(i)

                for it in range(5):
                    if it == 4:
                        proj_kv16()
                    else:
                        excl = it >= 1
                        xc = get_xc(it)
                        if excl:
                            ktmp = st_pool.tile([P, 8, CH], f8, tag="ktmp",
                                                name=f"ktmp{it}")
                            vtmp = st_pool.tile([P, 2, 2, D], f8, tag="vtmp",
                                                name=f"vtmp{it}")
                        for o in range(8):
                            ps = ps_pool.tile([P, CH], f32, tag="ps", name="psk")
                            for d2 in range(4):
                                nc.tensor.matmul(
                                    ps[:],
                                    lhsT=wk8[:, ds(2 * d2, 2), ts(o, P)],
                                    rhs=xc[:, ds(2 * d2, 2), :],
                                    start=(d2 == 0),
                                    stop=(d2 == 3),
                                    perf_mode=DR,
                                )
                            kdst = (
                                ktmp[:, o, :] if excl
                                else KTp[0][:, o, ds(CH, CH)]
                            )
                            if o % 2 == 0:
                                nc.vector.tensor_copy(kdst, ps[:])
                            else:
                                nc.scalar.copy(kdst, ps[:])
                        get_xc(it + 1)
                        if it == 0:
                            # deferred loads, enqueued between chunk-1's K and
                            # V work (wv8 must precede the V copies in the
                            # ACT queue to avoid a trigger deadlock)
                            for d2 in range(4):
                                nc.scalar.dma_start(
                                    out=wv8[:, ds(2 * d2, 2), :],
                                    in_=wv8d[:, ds(2 * d2, 2), :],
                                )
                            nc.scalar.dma_start(out=wq8[:], in_=wq8d[:])
                            xq_pending[1] = load_xq(1)
                            xq_pending[2] = load_xq(2)
                            for sp in range(4):
                                eng = nc.gpsimd if sp % 2 == 0 else nc.scalar
                                eng.dma_start(
                                    out=wa16[:, ds(2 * sp, 2), :],
                                    in_=wq16d[:, ds(2 * sp, 2), :],
                                )
                            nc.sync.dma_start(out=x16q[:], in_=xq16d[:])
                        for kt_i in range(4):
                            for oh in range(2):
                                ps = ps_pool.tile([P, CH], f32, tag="ps", name="psv")
                                for d2 in range(4):
                                    nc.tensor.matmul(
                                        ps[:],
                                        lhsT=xc[:, ds(2 * d2, 2), ts(kt_i, P)],
                                        rhs=wv8[:, ds(2 * d2, 2), ts(oh, CH)],
                                        start=(d2 == 0),
                                        stop=(d2 == 3),
                                        perf_mode=DR,
                                    )
                                vdst = (
                                    vtmp[:, kt_i // 2, kt_i % 2, ts(oh, CH)]
                                    if excl
                                    else vsb[:, 2 + kt_i // 2, kt_i % 2,
                                             ts(oh, CH)]
                                )
                                nc.scalar.copy(vdst, ps[:])
                        if excl:
                            li = it - 1
                            cin = cc_a_in if li < 2 else cc_b_in
                            sec = li if li < 2 else 0
                            nc.scalar.dma_start(out=cin[sec, 0], in_=ktmp[:])
                            nc.scalar.dma_start(out=cin[sec, 1], in_=vtmp[:])
                            if li == 1:
                                nc.gpsimd.collective_compute(
                                    "AllGather", mybir.AluOpType.bypass,
                                    replica_groups=CC_GROUPS,
                                    ins=[cc_a_in[:]], outs=[cc_a_out[:]],
                                )
                            elif li == 2:
                                nc.gpsimd.collective_compute(
                                    "AllGather", mybir.AluOpType.bypass,
                                    replica_groups=CC_GROUPS,
                                    ins=[cc_b_in[:]], outs=[cc_b_out[:]],
                                )
                    sq = q_sched.get(it)
                    if sq is not None:
                        if sq == 0:
                            proj_q16()
                            # island loads (consumed at it==4); wa16 reuses
                            # wq16's tile (WAR resolves as proj_q16 drains)
                            for sp in range(4):
                                eng = nc.sync if sp % 2 == 0 else nc.gpsimd
                                eng.dma_start(
                                    out=wa16[:, ds(2 * sp, 2), :],
                                    in_=wk16d[:, ds(2 * sp, 2), :],
                                )
                            nc.sync.dma_start(out=x16c[:], in_=xc16d[:])
                            nc.gpsimd.dma_start(out=xh8[:], in_=xh8d[:])
                        else:
                            proj_q8(sq)
                            if sq == 1:
                                for sp in range(4):
                                    nc.scalar.dma_start(
                                        out=wv16[:, ds(2 * sp, 2), :],
                                        in_=wv16d[:, ds(2 * sp, 2), :],
                                    )
                            if sq == 2:
                                xq_pending[3] = load_xq(3)
                # read the gathered exclusive sections back (both ranks'),
                # at chunk-uniform addresses; DMAs wait on the collectives
                qi = 0
                for r in range(2):
                    for s2 in range(2):
                        cid = CH_A[r][s2]
                        eng = nc.gpsimd if qi % 2 == 0 else nc.sync
                        eng.dma_start(
                            out=KTp[cid // 2][:, :, ds((cid % 2) * CH, CH)],
                            in_=cc_a_out[r, s2, 0],
                        )
                        eng.dma_start(
                            out=vsb[:, ds(2 * cid, 2), :, :],
                            in_=cc_a_out[r, s2, 1],
                        )
                        qi += 1
                for r in range(2):
                    cid = CH_B[r]
                    eng = nc.gpsimd if qi % 2 == 0 else nc.sync
                    eng.dma_start(
                        out=KTp[cid // 2][:, :, ds((cid % 2) * CH, CH)],
                        in_=cc_b_out[r, 0, 0],
                    )
                    eng.dma_start(
                        out=vsb[:, ds(2 * cid, 2), :, :],
                        in_=cc_b_out[r, 0, 1],
                    )
                    qi += 1

            # ---------------- Phase 2: attention ---------------------------
            # Per slot: all score k-tiles (P tiles + den accumulate), then
            # ctx as ONE psum group per o; ft multiplies read psum directly.
            # Tiles causally full on BOTH halves skip the mask STT: ACT
            # writes exp() straight into the fp8 P pair tile.
            CLEAN_NK = [
                min(4 * CHUNKS_H[0][s], 4 * CHUNKS_H[1][s], NK[s])
                for s in range(NSLOT)
            ]
            with (
                tc.tile_pool(name="pt", bufs=20) as p_pool,
                tc.tile_pool(name="p6", bufs=4) as p16_pool,
                tc.tile_pool(name="et", bufs=3) as e_pool,
                tc.tile_pool(name="fo", bufs=6) as f_pool,
                tc.tile_pool(name="dsb", bufs=2) as den_pool,
                tc.tile_pool(name="cx0", bufs=1) as ctx0_pool,
                tc.tile_pool(name="pss", bufs=4, space="PSUM") as s_ps_pool,
                tc.tile_pool(name="psc", bufs=3, space="PSUM") as c_ps_pool,
                tc.tile_pool(name="psb", bufs=1, space="PSUM") as b_ps_pool,
            ):
                for slot in range(NSLOT):
                    nk = NK[slot]
                    acc = den_pool.tile([P, CH], f16, tag="acc", name="acc")
                    pt16s = []
                    pt2s = []
                    cur_pt2 = None
                    for j in range(nk):
                        island = slot == 0 and j < 2
                        clean = j < CLEAN_NK[slot]
                        sps = s_ps_pool.tile([P, CH], f32, name="sps")
                        if island:
                            for o in range(8):
                                nc.tensor.matmul(
                                    sps[:],
                                    lhsT=KT16[:, o, ds(j * P, P)],
                                    rhs=QT16[:, o, :],
                                    start=(o == 0),
                                    stop=(o == 7),
                                )
                        else:
                            for o2 in range(4):
                                nc.tensor.matmul(
                                    sps[:],
                                    lhsT=KTp[j // 8][
                                        :, ds(2 * o2, 2), ds((j % 8) * P, P)
                                    ],
                                    rhs=QTs[slot][:, ds(2 * o2, 2), :],
                                    start=(o2 == 0),
                                    stop=(o2 == 3),
                                    perf_mode=DR,
                                )
                        if not island and j % 2 == 0:
                            cur_pt2 = p_pool.tile([P, 2, CH], f8, tag="pt",
                                                  name="pt2")
                        col = SLOTBASE[slot] + j
                        if island:
                            et = e_pool.tile([P, CH], f16, tag="et", name="et")
                            nc.scalar.activation(et[:], sps[:], Exp, scale=SCALE)
                            pt = p16_pool.tile([P, CH], f16, tag="pt16",
                                               name="pt16")
                            nc.vector.scalar_tensor_tensor(
                                out=pt[:],
                                in0=dmat_sb[:],
                                scalar=amat_sb[:, ds(col, 1)],
                                in1=et[:],
                                op0=is_le,
                                op1=mult,
                            )
                            pt16s.append(pt)
                            ptv = pt[:]
                        elif clean:
                            nc.scalar.activation(
                                cur_pt2[:, j % 2, :], sps[:], Exp, scale=SCALE
                            )
                            ptv = cur_pt2[:, j % 2, :]
                        else:
                            et = e_pool.tile([P, CH], f16, tag="et", name="et")
                            nc.scalar.activation(et[:], sps[:], Exp, scale=SCALE)
                            nc.vector.scalar_tensor_tensor(
                                out=cur_pt2[:, j % 2, :],
                                in0=dmat_sb[:],
                                scalar=amat_sb[:, ds(col, 1)],
                                in1=et[:],
                                op0=is_le,
                                op1=mult,
                            )
                            ptv = cur_pt2[:, j % 2, :]
                        if not island and j % 2 == 1:
                            pt2s.append(cur_pt2)
                        if j == 0:
                            nc.vector.tensor_copy(acc[:], ptv)
                        else:
                            nc.vector.tensor_add(acc[:], acc[:], ptv)
                    # den -> 1/den = exp(-ln(den)) -> broadcast; the o=0 ctx
                    # group is emitted between den and bps so the PE never
                    # waits on the ACT chain.
                    dpsf = b_ps_pool.tile([P, CH], f32, tag="db", name="dps")
                    dps = dpsf[0:1, :]
                    nc.tensor.matmul(
                        dps, lhsT=ones_k_sb[:], rhs=acc[:], start=True,
                        stop=True,
                    )
                    lden = f_pool.tile([1, CH], f32, tag="lden", name="lden")
                    nc.scalar.activation(lden[:], dps, Ln)
                    den_sb = f_pool.tile([1, CH], f16, tag="den", name="den")
                    nc.scalar.activation(den_sb[:], lden[:], Exp, scale=-1.0)

                    n_mm = len(pt16s) + len(pt2s)

                    def ctx_group(o, slot=slot, n_mm=n_mm, pt16s=pt16s,
                                  pt2s=pt2s):
                        cps = c_ps_pool.tile([P, CH], f32, name="cps")
                        idx = 0
                        for jj, pt in enumerate(pt16s):
                            nc.tensor.matmul(
                                cps[:],
                                lhsT=v16[:, jj, ts(o, P)],
                                rhs=pt[:],
                                start=(idx == 0),
                                stop=(idx == n_mm - 1),
                            )
                            idx += 1
                        p0 = 1 if slot == 0 else 0
                        for pi, pt2 in enumerate(pt2s):
                            nc.tensor.matmul(
                                cps[:],
                                lhsT=vsb[:, p0 + pi, :, ts(o, P)],
                                rhs=pt2[:],
                                start=(idx == 0),
                                stop=(idx == n_mm - 1),
                                perf_mode=DR,
                            )
                            idx += 1
                        return cps

                    if slot == 0:
                        # slot 0's ctx is short (6-MM groups): flush all 8
                        # groups to SBUF via DVE (no rec dependency, PSUM
                        # frees immediately) = 9.6us of pure PE cover for
                        # the den->bps ACT chain, instead of stalling at bps
                        ctx_sb = ctx0_pool.tile([P, 8, CH], f32, tag="ctx0",
                                                name="ctx0")
                        for o in range(8):
                            cps = ctx_group(o)
                            nc.vector.tensor_copy(ctx_sb[:, o, :], cps[:])
                        bps = b_ps_pool.tile([P, CH], f32, tag="db", name="bps")
                        nc.tensor.matmul(
                            bps[:], lhsT=ones_r_sb[:], rhs=den_sb[:],
                            start=True, stop=True,
                        )
                        rec = f_pool.tile([P, CH], f16, tag="rec", name="rec")
                        nc.scalar.copy(rec[:], bps[:])
                        for o in range(8):
                            ft = f_pool.tile([P, CH], f16, tag="ft", name="ft")
                            nc.vector.tensor_mul(ft[:], ctx_sb[:, o, :], rec[:])
                            eng = nc.sync if o % 2 == 0 else nc.scalar
                            eng.dma_start(
                                out=outT[ds(o * P, P), ts(slot, CH)],
                                in_=ft[:],
                            )
                        continue
                    head = [(o, ctx_group(o)) for o in range(3)]
                    bps = b_ps_pool.tile([P, CH], f32, tag="db", name="bps")
                    nc.tensor.matmul(
                        bps[:], lhsT=ones_r_sb[:], rhs=den_sb[:], start=True,
                        stop=True,
                    )
                    rec = f_pool.tile([P, CH], f16, tag="rec", name="rec")
                    nc.scalar.copy(rec[:], bps[:])
                    for o, cps in head:
                        ft = f_pool.tile([P, CH], f16, tag="ft", name="ft")
                        nc.vector.tensor_mul(ft[:], cps[:], rec[:])
                        eng = nc.sync if o % 2 == 0 else nc.scalar
                        eng.dma_start(
                            out=outT[ds(o * P, P), ts(slot, CH)], in_=ft[:]
                        )
                    for o in range(3, 8):
                        cps = ctx_group(o)
                        ft = f_pool.tile([P, CH], f16, tag="ft", name="ft")
                        nc.vector.tensor_mul(ft[:], cps[:], rec[:])
                        eng = nc.sync if o % 2 == 0 else nc.scalar
                        eng.dma_start(
                            out=outT[ds(o * P, P), ts(slot, CH)], in_=ft[:]
                        )

    nc.compile()
    return nc


def _get_program():
    global _PROGRAM
    if _PROGRAM is None:
        _PROGRAM = _build_program()
    return _PROGRAM


def _make_in_maps(x, W_query, W_key, W_value):
    import ml_dtypes

    f8 = ml_dtypes.float8_e4m3

    xT = np.ascontiguousarray(
        np.asarray(x, dtype=np.float32).transpose(0, 2, 1).astype(np.float16)
    )
    xT8 = xT.astype(f8)

    def tile_w(w, dt):
        # [d, o] -> [p, d_slab, o]
        wt = np.asarray(w, dtype=np.float32).T.astype(np.float16).astype(dt)
        return np.ascontiguousarray(wt.reshape(8, P, D).transpose(1, 0, 2))

    def tile_x(xt, nch):
        # [d, s] -> [chunk, p, d_slab, s_off]
        return np.ascontiguousarray(
            xt.reshape(8, P, nch, CH).transpose(2, 1, 0, 3)
        )

    w8 = {k: tile_w(w, f8) for k, w in
          (("wq8", W_query), ("wk8", W_key), ("wv8", W_value))}
    w16 = {k: tile_w(w, np.float16) for k, w in
           (("wq16", W_query), ("wk16", W_key), ("wv16", W_value))}
    dmat = (
        np.arange(P, dtype=np.float32)[:, None]
        - np.arange(CH, dtype=np.float32)[None, :]
    )
    dmat = np.ascontiguousarray(dmat.astype(np.float16))
    amat_h = []
    for h in range(2):
        a = np.zeros((P, 80), np.float16)
        for slot in range(NSLOT):
            cid = CHUNKS_H[h][slot]
            for j in range(NK[slot]):
                a[:, SLOTBASE[slot] + j] = CH * cid - P * j
        amat_h.append(a)
    ones_k = np.ones((P, 1), np.float16)
    ones_r = np.ones((1, P), np.float16)

    in_maps = []
    for core in range(8):
        b, h = core // 2, core % 2
        chunks = CHUNKS_H[h]
        xq8 = np.stack(
            [tile_x(xT8[b][:, c * CH : (c + 1) * CH], 1)[0] for c in chunks[1:]]
        )
        xq16 = tile_x(
            xT[b][:, chunks[0] * CH : (chunks[0] + 1) * CH], 1
        )[0]
        # projection positions: chunk 1 then this core's exclusive chunks
        pos_chunks = [1] + ([2, 3, 4] if h == 0 else [5, 6, 7])
        xT8_core = np.stack(
            [tile_x(xT8[b][:, c * CH : (c + 1) * CH], 1)[0] for c in pos_chunks]
        )
        in_maps.append(
            {
                "xT8": xT8_core,
                "xc16": np.ascontiguousarray(
                    xT[b][:, :256].reshape(8, P, 256).transpose(1, 0, 2)
                ),
                "xh8": np.ascontiguousarray(
                    xT8[b][:, 256:512].reshape(8, P, 256).transpose(1, 0, 2)
                ),
                "xq8": xq8,
                "xq16": xq16,
                **w8,
                **w16,
                "amat": amat_h[h],
                "dmat": dmat,
                "ones_k": ones_k,
                "ones_r": ones_r,
            }
        )
    return in_maps


def _assemble(results):
    out = np.empty((B, S, D), np.float32)
    for core in range(8):
        b, h = core // 2, core % 2
        oT = np.asarray(results[core]["outT"]).astype(np.float32)  # [D, NQ]
        for slot, c in enumerate(CHUNKS_H[h]):
            out[b, c * CH : (c + 1) * CH, :] = oT[:, slot * CH : (slot + 1) * CH].T
    return out


def run(inputs, trace=False, trace_cores=None):
    """Run the kernel; returns (output, BassKernelResults)."""
    from concourse.bass_utils import run_bass_kernel_spmd

    nc = _get_program()
    in_maps = _make_in_maps(
        inputs["x"], inputs["W_query"], inputs["W_key"], inputs["W_value"]
    )
    kw = {}
    if trace:
        kw = dict(trace=True, trace_cores=trace_cores, stitch_traces=False)
    res = run_bass_kernel_spmd(nc, in_maps, list(range(8)), **kw)
    return _assemble(res.results), res


def kernel(x, W_query, W_key, W_value):
    out, _ = run({"x": x, "W_query": W_query, "W_key": W_key, "W_value": W_value})
    return out


# revision 45
# speedup vs baseline: 1.0299x; 1.0024x over previous
"""Causal single-head attention (B=4, S=4096, D=1024) on 8 TRN2 NeuronCores.

Sharding: core = (batch b, half h).  Each core computes attention output for
2048 queries of one batch: query chunks {0,3,4,7} (h=0) or {1,2,5,6} (h=1) of
8x512, which balances causal work.  Each core projects K^T/V for its full
batch (Q projections zippered in between the chunks); K^T and V both live
entirely in SBUF as fp8 (V is 4MB = 32KB/partition), so phase 2 needs no DMA
except the output.

All heavy matmuls run fp8e4m3 with perf_mode=DoubleRow (2 contraction slabs
per pass), except a small fp16 "island" covering keys 0..511 for slot-0
queries (chunks 0/1): early causal queries have peaked softmax, so fp8
quantization of scores/V would land directly on the output there.  The
island chunk (c=0) is projected LAST so its fp16 weights never gate startup.

  K^T/Q^T/V projections:  psum = sum_d2 WT[d2,:,:128].T @ x^T[d2,:,:]  (fp8 DR)
  scores^T[k,q]        :  psum = sum_o2 KT[o2,:,k128].T @ QT[o2,:,q512] (fp8 DR)
  P = exp(s*scale) * causal_mask   (mask = (iota_k - iota_q) <= a[slot,j]);
      causally-full tiles skip the mask: ACT writes exp straight to fp8
  den[1,q]             :  DVE-accumulate P tiles, then ones[k,1].T @ acc;
                          1/den = exp(-ln(den)) on ACT
  ctx^T[o,q]           :  one psum group per (slot,o) over all k-pairs
                          (fp8 DR); ft = psum * (1/den) -> fp16 out
"""

import sys

for _p in ("/opt/trn_rl_repo",):
    if _p not in sys.path:
        sys.path.insert(0, _p)

import numpy as np

B, S, D = 4, 4096, 1024
P = 128
CH = 512                       # query chunk
NSLOT = 4                      # chunks per core
NQ = NSLOT * CH                # queries per core
NK = [8, 16, 24, 32]           # k-tiles per slot (uniform across cores)
SLOTBASE = [0, 8, 24, 48]      # amat column base per slot
CHUNKS_H = [[0, 3, 4, 7], [1, 2, 5, 6]]
SCALE = 1.0 / 32.0             # 1/sqrt(D)

_PROGRAM = None


def _build_program():
    import concourse.bass as bass
    import concourse.tile as tile
    import concourse.mybir as mybir
    from concourse import bacc
    from concourse.bass import ds, ts

    f32 = mybir.dt.float32
    f16 = mybir.dt.float16
    f8 = mybir.dt.float8e4
    DR = mybir.MatmulPerfMode.DoubleRow

    nc = bacc.Bacc(trn_type="TRN2", target_bir_lowering=False, debug=False,
                   num_devices=8)

    # positions: [chunk 1, excl0, excl1, excl2] (excl = 2,3,4 on h=0 /
    # 5,6,7 on h=1); chunk 0 arrives as fp16 via xc16
    xT8 = nc.declare_dram_parameter("xT8", [4, P, 8, CH], f8, isOutput=False)
    xc16d = nc.declare_dram_parameter("xc16", [P, 8, 256], f16, isOutput=False)
    xh8d = nc.declare_dram_parameter("xh8", [P, 8, 256], f8, isOutput=False)
    xq8d = nc.declare_dram_parameter("xq8", [3, P, 8, CH], f8, isOutput=False)
    xq16d = nc.declare_dram_parameter("xq16", [P, 8, CH], f16, isOutput=False)
    wq8d = nc.declare_dram_parameter("wq8", [P, 8, D], f8, isOutput=False)
    wk8d = nc.declare_dram_parameter("wk8", [P, 8, D], f8, isOutput=False)
    wv8d = nc.declare_dram_parameter("wv8", [P, 8, D], f8, isOutput=False)
    wq16d = nc.declare_dram_parameter("wq16", [P, 8, D], f16, isOutput=False)
    wk16d = nc.declare_dram_parameter("wk16", [P, 8, D], f16, isOutput=False)
    wv16d = nc.declare_dram_parameter("wv16", [P, 8, D], f16, isOutput=False)
    amat = nc.declare_dram_parameter("amat", [P, 80], f16, isOutput=False)
    dmat = nc.declare_dram_parameter("dmat", [P, CH], f16, isOutput=False)
    ones_k = nc.declare_dram_parameter("ones_k", [P, 1], f16, isOutput=False)
    ones_r = nc.declare_dram_parameter("ones_r", [1, P], f16, isOutput=False)
    outT = nc.declare_dram_parameter("outT", [D, NQ], f16, isOutput=True)

    H = S // 4  # 1024: columns per resident K^T piece

    # pairwise exchange buffers for the exclusive-chunk K/V split.
    # Section = 4KB/partition: K chunk [P, 8, CH] or V chunk [P, 2, 2, D].
    SEC = 8 * CH
    cc_a_in = nc.dram_tensor("cc_a_in", [2, 2, P, SEC], f8)
    cc_a_out = nc.dram_tensor("cc_a_out", [2, 2, 2, P, SEC], f8)
    cc_b_in = nc.dram_tensor("cc_b_in", [1, 2, P, SEC], f8)
    cc_b_out = nc.dram_tensor("cc_b_out", [2, 1, 2, P, SEC], f8)
    CC_GROUPS = [[0, 1], [2, 3], [4, 5], [6, 7]]
    CH_A = [[2, 3], [5, 6]]   # cc_a sections: rank -> chunk ids
    CH_B = [4, 7]             # cc_b: rank -> chunk id

    Exp = mybir.ActivationFunctionType.Exp
    Ln = mybir.ActivationFunctionType.Ln
    is_le = mybir.AluOpType.is_le
    mult = mybir.AluOpType.mult

    with tile.TileContext(nc, pool_alloc_mode="queue") as tc:
        with (
            tc.tile_pool(name="kt", bufs=1) as kt_pool,
            tc.tile_pool(name="qt", bufs=1) as qt_pool,
            tc.tile_pool(name="vs", bufs=1) as vs_pool,
            tc.tile_pool(name="const", bufs=1) as const_pool,
        ):
            KTp = [
                kt_pool.tile([P, 8, H], f8, tag=f"kt{i}", name=f"KTp{i}")
                for i in range(4)
            ]
            KT16 = kt_pool.tile([P, 8, 256], f16, tag="kt16", name="KT16")
            QTs = [
                qt_pool.tile([P, 8, CH], f8, tag=f"qt{i}", name=f"QTs{i}")
                for i in range(NSLOT)
            ]
            QT16 = qt_pool.tile([P, 8, CH], f16, tag="qt16", name="QT16")
            # V resident in SBUF: fp8 k-pair layout + fp16 island (keys 0..511)
            vsb = vs_pool.tile([P, 16, 2, D], f8, tag="vsb", name="vsb")
            v16 = vs_pool.tile([P, 2, D], f16, tag="v16", name="v16")
            dmat_sb = const_pool.tile([P, CH], f16, tag="dmat")
            amat_sb = const_pool.tile([P, 80], f16, tag="amat")
            ones_k_sb = const_pool.tile([P, 1], f16, tag="onesk")
            ones_r_sb = const_pool.tile([1, P], f16, tag="onesr")
            nc.gpsimd.dma_start(out=dmat_sb[:], in_=dmat[:])
            nc.gpsimd.dma_start(out=amat_sb[:], in_=amat[:])
            nc.gpsimd.dma_start(out=ones_k_sb[:], in_=ones_k[:])
            nc.gpsimd.dma_start(out=ones_r_sb[:], in_=ones_r[:])

            # ---------- Phase 0+1: local projections (K, V, Q zippered) ----
            with (
                tc.tile_pool(name="w0", bufs=1) as w_pool,
                tc.tile_pool(name="xc", bufs=2) as x_pool,
                tc.tile_pool(name="xq", bufs=2) as xq_pool,
                tc.tile_pool(name="st", bufs=2) as st_pool,
                tc.tile_pool(name="ps0", bufs=4, space="PSUM") as ps_pool,
            ):
                wk8 = w_pool.tile([P, 8, D], f8, tag="wk8")
                wv8 = w_pool.tile([P, 8, D], f8, tag="wv8")
                wq8 = w_pool.tile([P, 8, D], f8, tag="wq8")
                # wa16 carries wq16 (Q island) then is reloaded with wk16
                wa16 = w_pool.tile([P, 8, D], f16, tag="wa16")
                wv16 = w_pool.tile([P, 8, D], f16, tag="wv16")
                x16q = w_pool.tile([P, 8, CH], f16, tag="x16q")
                x16c = w_pool.tile([P, 8, 256], f16, tag="x16c")
                xh8 = w_pool.tile([P, 8, 256], f8, tag="xh8")
                # striped initial loads for the first fp8 chunk
                for d2 in range(4):
                    eng = nc.sync if d2 < 2 else nc.scalar
                    eng.dma_start(
                        out=wk8[:, ds(2 * d2, 2), :],
                        in_=wk8d[:, ds(2 * d2, 2), :],
                    )


                def load_xq(s):
                    xq = xq_pool.tile([P, 8, CH], f8, tag="xq", name=f"xq{s}")
                    nc.scalar.dma_start(out=xq[:], in_=xq8d[s - 1])
                    return xq

                xq_pending = {}

                def proj_q8(s):
                    xq = xq_pending[s]
                    for o in range(8):
                        ps = ps_pool.tile([P, CH], f32, tag="ps", name="psq")
                        for d2 in range(4):
                            nc.tensor.matmul(
                                ps[:],
                                lhsT=wq8[:, ds(2 * d2, 2), ts(o, P)],
                                rhs=xq[:, ds(2 * d2, 2), :],
                                start=(d2 == 0),
                                stop=(d2 == 3),
                                perf_mode=DR,
                            )
                        nc.vector.tensor_copy(QTs[s][:, o, :], ps[:])

                def proj_q16():
                    # slot-0 Q in fp16 (wa16 = wq16, x16 = xq16), dual-cast
                    for o in range(8):
                        ps = ps_pool.tile([P, CH], f32, tag="ps", name="psq6")
                        for d in range(8):
                            nc.tensor.matmul(
                                ps[:],
                                lhsT=wa16[:, d, ts(o, P)],
                                rhs=x16q[:, d, :],
                                start=(d == 0),
                                stop=(d == 7),
                            )
                        nc.vector.tensor_copy(QT16[:, o, :], ps[:])
                        nc.scalar.copy(QTs[0][:, o, :], ps[:])

                def proj_kv16():
                    # chunk 0: tokens 0..255 in fp16 (wa16 = wk16, x16c),
                    # tokens 256..511 in fp8 (xh8) - the island only needs
                    # keys 0..255
                    for o in range(8):
                        ps = ps_pool.tile([P, 256], f32, tag="ps6", name="psk6")
                        for d in range(8):
                            nc.tensor.matmul(
                                ps[:],
                                lhsT=wa16[:, d, ts(o, P)],
                                rhs=x16c[:, d, :],
                                start=(d == 0),
                                stop=(d == 7),
                            )
                        nc.vector.tensor_copy(KT16[:, o, :], ps[:])
                        nc.scalar.copy(KTp[0][:, o, ds(0, 256)], ps[:])
                    for kt_i in range(2):
                        for oh in range(2):
                            ps = ps_pool.tile([P, CH], f32, tag="ps", name="psv6")
                            for d in range(8):
                                nc.tensor.matmul(
                                    ps[:],
                                    lhsT=x16c[:, d, ts(kt_i, P)],
                                    rhs=wv16[:, d, ts(oh, CH)],
                                    start=(d == 0),
                                    stop=(d == 7),
                                )
                            nc.scalar.copy(v16[:, kt_i, ts(oh, CH)], ps[:])
                            nc.vector.tensor_copy(
                                vsb[:, 0, kt_i, ts(oh, CH)], ps[:]
                            )
                    for o in range(8):
                        ps = ps_pool.tile([P, 256], f32, tag="ps6", name="psk6b")
                        for d2 in range(4):
                            nc.tensor.matmul(
                                ps[:],
                                lhsT=wk8[:, ds(2 * d2, 2), ts(o, P)],
                                rhs=xh8[:, ds(2 * d2, 2), :],
                                start=(d2 == 0),
                                stop=(d2 == 3),
                                perf_mode=DR,
                            )
                            if o % 2 == 0 and d2 == 3:
                                nc.vector.tensor_copy(
                                    KTp[0][:, o, ds(256, 256)], ps[:])
                            elif d2 == 3:
                                nc.scalar.copy(
                                    KTp[0][:, o, ds(256, 256)], ps[:])
                    for kt_i in range(2):
                        for oh in range(2):
                            ps = ps_pool.tile([P, CH], f32, tag="ps", name="psv6b")
                            for d2 in range(4):
                                nc.tensor.matmul(
                                    ps[:],
                                    lhsT=xh8[:, ds(2 * d2, 2), ts(kt_i, P)],
                                    rhs=wv8[:, ds(2 * d2, 2), ts(oh, CH)],
                                    start=(d2 == 0),
                                    stop=(d2 == 3),
                                    perf_mode=DR,
                                )
                            nc.scalar.copy(
                                vsb[:, 1, kt_i, ts(oh, CH)], ps[:]
                            )

                # positions: [chunk1, excl0, excl1, excl2, chunk0].  Chunk 0
                # (fp16 island) LAST so its weights stream in while fp8
                # chunks compute.  Exclusive chunks (this core only) are
                # staged to DRAM and AllGathered with the pair core; both
                # cores then read all six sections back into KTp/vsb at
                # uniform addresses.  Q slots zippered after iters 1..4
                # (slot 0 at it==3 so x16/wa16 can be reloaded for chunk 0).
                q_sched = {1: 0, 2: 1, 3: 2, 4: 3}
                xc_tiles = {}

                def get_xc(i):
                    if i not in xc_tiles and i < 4:
                        xc_tiles[i] = x_pool.tile([P, 8, CH], f8, tag="xc",
                                                  name=f"xc{i}")
                        if i == 0:
                            for sp in range(4):
                                eng = nc.gpsimd if sp % 2 == 0 else nc.sync
                                eng.dma_start(
                                    out=xc_tiles[i][:, ds(sp * 2, 2), :],
                                    in_=xT8[0][:, ds(sp * 2, 2), :],
                                )
                        else:
                            nc.sync.dma_start(out=xc_tiles[i][:], in_=xT8[i])
                    return xc_tiles.get(i)

                for it in range(5):
                    if it == 4:
                        proj_kv16()
                    else:
                        excl = it >= 1
                        xc = get_xc(it)
                        if excl:
                            ktmp = st_pool.tile([P, 8, CH], f8, tag="ktmp",
                                                name=f"ktmp{it}")
                            vtmp = st_pool.tile([P, 2, 2, D], f8, tag="vtmp",
                                                name=f"vtmp{it}")
                        for o in range(8):
                            ps = ps_pool.tile([P, CH], f32, tag="ps", name="psk")
                            for d2 in range(4):
                                nc.tensor.matmul(
                                    ps[:],
                                    lhsT=wk8[:, ds(2 * d2, 2), ts(o, P)],
                                    rhs=xc[:, ds(2 * d2, 2), :],
                                    start=(d2 == 0),
                                    stop=(d2 == 3),
                                    perf_mode=DR,
                                )
                            kdst = (
                                ktmp[:, o, :] if excl
                                else KTp[0][:, o, ds(CH, CH)]
                            )
                            if o % 2 == 0:
                                nc.vector.tensor_copy(kdst, ps[:])
                            else:
                                nc.scalar.copy(kdst, ps[:])
                        get_xc(it + 1)
                        if it == 0:
                            # deferred loads, enqueued between chunk-1's K and
                            # V work (wv8 must precede the V copies in the
                            # ACT queue to avoid a trigger deadlock)
                            for d2 in range(4):
                                nc.scalar.dma_start(
                                    out=wv8[:, ds(2 * d2, 2), :],
                                    in_=wv8d[:, ds(2 * d2, 2), :],
                                )
                            nc.scalar.dma_start(out=wq8[:], in_=wq8d[:])
                            xq_pending[1] = load_xq(1)
                            xq_pending[2] = load_xq(2)
                            for sp in range(4):
                                eng = nc.gpsimd if sp % 2 == 0 else nc.scalar
                                eng.dma_start(
                                    out=wa16[:, ds(2 * sp, 2), :],
                                    in_=wq16d[:, ds(2 * sp, 2), :],
                                )
                            nc.sync.dma_start(out=x16q[:], in_=xq16d[:])
                        for kt_i in range(4):
                            for oh in range(2):
                                ps = ps_pool.tile([P, CH], f32, tag="ps", name="psv")
                                for d2 in range(4):
                                    nc.tensor.matmul(
                                        ps[:],
                                        lhsT=xc[:, ds(2 * d2, 2), ts(kt_i, P)],
                                        rhs=wv8[:, ds(2 * d2, 2), ts(oh, CH)],
                                        start=(d2 == 0),
                                        stop=(d2 == 3),
                                        perf_mode=DR,
                                    )
                                vdst = (
                                    vtmp[:, kt_i // 2, kt_i % 2, ts(oh, CH)]
                                    if excl
                                    else vsb[:, 2 + kt_i // 2, kt_i % 2,
                                             ts(oh, CH)]
                                )
                                nc.scalar.copy(vdst, ps[:])
                        if excl:
                            li = it - 1
                            cin = cc_a_in if li < 2 else cc_b_in
                            sec = li if li < 2 else 0
                            nc.scalar.dma_start(out=cin[sec, 0], in_=ktmp[:])
                            nc.scalar.dma_start(out=cin[sec, 1], in_=vtmp[:])
                            if li == 1:
                                nc.gpsimd.collective_compute(
                                    "AllGather", mybir.AluOpType.bypass,
                                    replica_groups=CC_GROUPS,
                                    ins=[cc_a_in[:]], outs=[cc_a_out[:]],
                                )
                            elif li == 2:
                                nc.gpsimd.collective_compute(
                                    "AllGather", mybir.AluOpType.bypass,
                                    replica_groups=CC_GROUPS,
                                    ins=[cc_b_in[:]], outs=[cc_b_out[:]],
                                )
                    sq = q_sched.get(it)
                    if sq is not None:
                        if sq == 0:
                            proj_q16()
                            # island loads (consumed at it==4); wa16 reuses
                            # wq16's tile (WAR resolves as proj_q16 drains)
                            for sp in range(4):
                                eng = nc.sync if sp % 2 == 0 else nc.gpsimd
                                eng.dma_start(
                                    out=wa16[:, ds(2 * sp, 2), :],
                                    in_=wk16d[:, ds(2 * sp, 2), :],
                                )
                            nc.sync.dma_start(out=x16c[:], in_=xc16d[:])
                            nc.gpsimd.dma_start(out=xh8[:], in_=xh8d[:])
                        else:
                            proj_q8(sq)
                            if sq == 1:
                                for sp in range(4):
                                    nc.scalar.dma_start(
                                        out=wv16[:, ds(2 * sp, 2), :],
                                        in_=wv16d[:, ds(2 * sp, 2), :],
                                    )
                            if sq == 2:
                                xq_pending[3] = load_xq(3)
                # read the gathered exclusive sections back (both ranks'),
                # at chunk-uniform addresses; DMAs wait on the collectives
                qi = 0
                for r in range(2):
                    for s2 in range(2):
                        cid = CH_A[r][s2]
                        eng = nc.gpsimd if qi % 2 == 0 else nc.sync
                        eng.dma_start(
                            out=KTp[cid // 2][:, :, ds((cid % 2) * CH, CH)],
                            in_=cc_a_out[r, s2, 0],
                        )
                        eng.dma_start(
                            out=vsb[:, ds(2 * cid, 2), :, :],
                            in_=cc_a_out[r, s2, 1],
                        )
                        qi += 1
                for r in range(2):
                    cid = CH_B[r]
                    eng = nc.gpsimd if qi % 2 == 0 else nc.sync
                    eng.dma_start(
                        out=KTp[cid // 2][:, :, ds((cid % 2) * CH, CH)],
                        in_=cc_b_out[r, 0, 0],
                    )
                    eng.dma_start(
                        out=vsb[:, ds(2 * cid, 2), :, :],
                        in_=cc_b_out[r, 0, 1],
                    )
                    qi += 1

            # ---------------- Phase 2: attention ---------------------------
            # Per slot: all score k-tiles (P tiles + den accumulate), then
            # ctx as ONE psum group per o; ft multiplies read psum directly.
            # Tiles causally full on BOTH halves skip the mask STT: ACT
            # writes exp() straight into the fp8 P pair tile.
            CLEAN_NK = [
                min(4 * CHUNKS_H[0][s], 4 * CHUNKS_H[1][s], NK[s])
                for s in range(NSLOT)
            ]
            with (
                tc.tile_pool(name="pt", bufs=20) as p_pool,
                tc.tile_pool(name="p6", bufs=4) as p16_pool,
                tc.tile_pool(name="et", bufs=3) as e_pool,
                tc.tile_pool(name="fo", bufs=6) as f_pool,
                tc.tile_pool(name="dsb", bufs=2) as den_pool,
                tc.tile_pool(name="cx0", bufs=1) as ctx0_pool,
                tc.tile_pool(name="pss", bufs=4, space="PSUM") as s_ps_pool,
                tc.tile_pool(name="psc", bufs=3, space="PSUM") as c_ps_pool,
                tc.tile_pool(name="psb", bufs=1, space="PSUM") as b_ps_pool,
            ):
                for slot in range(NSLOT):
                    nk = NK[slot]
                    acc = den_pool.tile([P, CH], f16, tag="acc", name="acc")
                    pt16s = []
                    pt2s = []
                    cur_pt2 = None
                    for j in range(nk):
                        island = slot == 0 and j < 2
                        clean = j < CLEAN_NK[slot]
                        sps = s_ps_pool.tile([P, CH], f32, name="sps")
                        if island:
                            for o in range(8):
                                nc.tensor.matmul(
                                    sps[:],
                                    lhsT=KT16[:, o, ds(j * P, P)],
                                    rhs=QT16[:, o, :],
                                    start=(o == 0),
                                    stop=(o == 7),
                                )
                        else:
                            for o2 in range(4):
                                nc.tensor.matmul(
                                    sps[:],
                                    lhsT=KTp[j // 8][
                                        :, ds(2 * o2, 2), ds((j % 8) * P, P)
                                    ],
                                    rhs=QTs[slot][:, ds(2 * o2, 2), :],
                                    start=(o2 == 0),
                                    stop=(o2 == 3),
                                    perf_mode=DR,
                                )
                        if not island and j % 2 == 0:
                            cur_pt2 = p_pool.tile([P, 2, CH], f8, tag="pt",
                                                  name="pt2")
                        col = SLOTBASE[slot] + j
                        if island:
                            et = e_pool.tile([P, CH], f16, tag="et", name="et")
                            nc.scalar.activation(et[:], sps[:], Exp, scale=SCALE)
                            pt = p16_pool.tile([P, CH], f16, tag="pt16",
                                               name="pt16")
                            nc.vector.scalar_tensor_tensor(
                                out=pt[:],
                                in0=dmat_sb[:],
                                scalar=amat_sb[:, ds(col, 1)],
                                in1=et[:],
                                op0=is_le,
                                op1=mult,
                            )
                            pt16s.append(pt)
                            ptv = pt[:]
                        elif clean:
                            nc.scalar.activation(
                                cur_pt2[:, j % 2, :], sps[:], Exp, scale=SCALE
                            )
                            ptv = cur_pt2[:, j % 2, :]
                        else:
                            et = e_pool.tile([P, CH], f16, tag="et", name="et")
                            nc.scalar.activation(et[:], sps[:], Exp, scale=SCALE)
                            nc.vector.scalar_tensor_tensor(
                                out=cur_pt2[:, j % 2, :],
                                in0=dmat_sb[:],
                                scalar=amat_sb[:, ds(col, 1)],
                                in1=et[:],
                                op0=is_le,
                                op1=mult,
                            )
                            ptv = cur_pt2[:, j % 2, :]
                        if not island and j % 2 == 1:
                            pt2s.append(cur_pt2)
                        if j == 0:
                            nc.vector.tensor_copy(acc[:], ptv)
                        else:
                            nc.vector.tensor_add(acc[:], acc[:], ptv)
                    # den -> 1/den = exp(-ln(den)) -> broadcast; the o=0 ctx
                    # group is emitted between den and bps so the PE never
                    # waits on the ACT chain.
                    dpsf = b_ps_pool.tile([P, CH], f32, tag="db", name="dps")
                    dps = dpsf[0:1, :]
                    nc.tensor.matmul(
                        dps, lhsT=ones_k_sb[:], rhs=acc[:], start=True,
                        stop=True,
                    )
                    lden = f_pool.tile([1, CH], f32, tag="lden", name="lden")
                    nc.scalar.activation(lden[:], dps, Ln)
                    den_sb = f_pool.tile([1, CH], f16, tag="den", name="den")
                    nc.scalar.activation(den_sb[:], lden[:], Exp, scale=-1.0)

                    n_mm = len(pt16s) + len(pt2s)

                    def ctx_group(o, slot=slot, n_mm=n_mm, pt16s=pt16s,
                                  pt2s=pt2s):
                        cps = c_ps_pool.tile([P, CH], f32, name="cps")
                        idx = 0
                        for jj, pt in enumerate(pt16s):
                            nc.tensor.matmul(
                                cps[:],
                                lhsT=v16[:, jj, ts(o, P)],
                                rhs=pt[:],
                                start=(idx == 0),
                                stop=(idx == n_mm - 1),
                            )
                            idx += 1
                        p0 = 1 if slot == 0 else 0
                        for pi, pt2 in enumerate(pt2s):
                            nc.tensor.matmul(
                                cps[:],
                                lhsT=vsb[:, p0 + pi, :, ts(o, P)],
                                rhs=pt2[:],
                                start=(idx == 0),
                                stop=(idx == n_mm - 1),
                                perf_mode=DR,
                            )
                            idx += 1
                        return cps

                    if slot == 0:
                        # slot 0's ctx is short (6-MM groups): flush all 8
                        # groups to SBUF via DVE (no rec dependency, PSUM
                        # frees immediately) = 9.6us of pure PE cover for
                        # the den->bps ACT chain, instead of stalling at bps
                        ctx_sb = ctx0_pool.tile([P, 8, CH], f32, tag="ctx0",
                                                name="ctx0")
                        for o in range(8):
                            cps = ctx_group(o)
                            nc.vector.tensor_copy(ctx_sb[:, o, :], cps[:])
                        bps = b_ps_pool.tile([P, CH], f32, tag="db", name="bps")
                        nc.tensor.matmul(
                            bps[:], lhsT=ones_r_sb[:], rhs=den_sb[:],
                            start=True, stop=True,
                        )
                        rec = f_pool.tile([P, CH], f16, tag="rec", name="rec")
                        nc.scalar.copy(rec[:], bps[:])
                        for o in range(8):
                            ft = f_pool.tile([P, CH], f16, tag="ft", name="ft")
                            nc.vector.tensor_mul(ft[:], ctx_sb[:, o, :], rec[:])
                            eng = nc.sync if o % 2 == 0 else nc.gpsimd
                            eng.dma_start(
                                out=outT[ds(o * P, P), ts(slot, CH)],
                                in_=ft[:],
                            )
                        continue
                    head = [(o, ctx_group(o)) for o in range(3)]
                    bps = b_ps_pool.tile([P, CH], f32, tag="db", name="bps")
                    nc.tensor.matmul(
                        bps[:], lhsT=ones_r_sb[:], rhs=den_sb[:], start=True,
                        stop=True,
                    )
                    rec = f_pool.tile([P, CH], f16, tag="rec", name="rec")
                    nc.scalar.copy(rec[:], bps[:])
                    for o, cps in head:
                        ft = f_pool.tile([P, CH], f16, tag="ft", name="ft")
                        nc.vector.tensor_mul(ft[:], cps[:], rec[:])
                        eng = nc.sync if o % 2 == 0 else nc.gpsimd
                        eng.dma_start(
                            out=outT[ds(o * P, P), ts(slot, CH)], in_=ft[:]
                        )
                    for o in range(3, 8):
                        cps = ctx_group(o)
                        ft = f_pool.tile([P, CH], f16, tag="ft", name="ft")
                        nc.vector.tensor_mul(ft[:], cps[:], rec[:])
                        eng = nc.sync if o % 2 == 0 else nc.gpsimd
                        eng.dma_start(
                            out=outT[ds(o * P, P), ts(slot, CH)], in_=ft[:]
                        )

    nc.compile()
    return nc


def _get_program():
    global _PROGRAM
    if _PROGRAM is None:
        _PROGRAM = _build_program()
    return _PROGRAM


def _make_in_maps(x, W_query, W_key, W_value):
    import ml_dtypes

    f8 = ml_dtypes.float8_e4m3

    xT = np.ascontiguousarray(
        np.asarray(x, dtype=np.float32).transpose(0, 2, 1).astype(np.float16)
    )
    xT8 = xT.astype(f8)

    def tile_w(w, dt):
        # [d, o] -> [p, d_slab, o]
        wt = np.asarray(w, dtype=np.float32).T.astype(np.float16).astype(dt)
        return np.ascontiguousarray(wt.reshape(8, P, D).transpose(1, 0, 2))

    def tile_x(xt, nch):
        # [d, s] -> [chunk, p, d_slab, s_off]
        return np.ascontiguousarray(
            xt.reshape(8, P, nch, CH).transpose(2, 1, 0, 3)
        )

    w8 = {k: tile_w(w, f8) for k, w in
          (("wq8", W_query), ("wk8", W_key), ("wv8", W_value))}
    w16 = {k: tile_w(w, np.float16) for k, w in
           (("wq16", W_query), ("wk16", W_key), ("wv16", W_value))}
    dmat = (
        np.arange(P, dtype=np.float32)[:, None]
        - np.arange(CH, dtype=np.float32)[None, :]
    )
    dmat = np.ascontiguousarray(dmat.astype(np.float16))
    amat_h = []
    for h in range(2):
        a = np.zeros((P, 80), np.float16)
        for slot in range(NSLOT):
            cid = CHUNKS_H[h][slot]
            for j in range(NK[slot]):
                a[:, SLOTBASE[slot] + j] = CH * cid - P * j
        amat_h.append(a)
    ones_k = np.ones((P, 1), np.float16)
    ones_r = np.ones((1, P), np.float16)

    in_maps = []
    for core in range(8):
        b, h = core // 2, core % 2
        chunks = CHUNKS_H[h]
        xq8 = np.stack(
            [tile_x(xT8[b][:, c * CH : (c + 1) * CH], 1)[0] for c in chunks[1:]]
        )
        xq16 = tile_x(
            xT[b][:, chunks[0] * CH : (chunks[0] + 1) * CH], 1
        )[0]
        # projection positions: chunk 1 then this core's exclusive chunks
        pos_chunks = [1] + ([2, 3, 4] if h == 0 else [5, 6, 7])
        xT8_core = np.stack(
            [tile_x(xT8[b][:, c * CH : (c + 1) * CH], 1)[0] for c in pos_chunks]
        )
        in_maps.append(
            {
                "xT8": xT8_core,
                "xc16": np.ascontiguousarray(
                    xT[b][:, :256].reshape(8, P, 256).transpose(1, 0, 2)
                ),
                "xh8": np.ascontiguousarray(
                    xT8[b][:, 256:512].reshape(8, P, 256).transpose(1, 0, 2)
                ),
                "xq8": xq8,
                "xq16": xq16,
                **w8,
                **w16,
                "amat": amat_h[h],
                "dmat": dmat,
                "ones_k": ones_k,
                "ones_r": ones_r,
            }
        )
    return in_maps


def _assemble(results):
    out = np.empty((B, S, D), np.float32)
    for core in range(8):
        b, h = core // 2, core % 2
        oT = np.asarray(results[core]["outT"]).astype(np.float32)  # [D, NQ]
        for slot, c in enumerate(CHUNKS_H[h]):
            out[b, c * CH : (c + 1) * CH, :] = oT[:, slot * CH : (slot + 1) * CH].T
    return out


def run(inputs, trace=False, trace_cores=None):
    """Run the kernel; returns (output, BassKernelResults)."""
    from concourse.bass_utils import run_bass_kernel_spmd

    nc = _get_program()
    in_maps = _make_in_maps(
        inputs["x"], inputs["W_query"], inputs["W_key"], inputs["W_value"]
    )
    kw = {}
    if trace:
        kw = dict(trace=True, trace_cores=trace_cores, stitch_traces=False)
    res = run_bass_kernel_spmd(nc, in_maps, list(range(8)), **kw)
    return _assemble(res.results), res


def kernel(x, W_query, W_key, W_value):
    out, _ = run({"x": x, "W_query": W_query, "W_key": W_key, "W_value": W_value})
    return out
